# revision 34
# baseline (speedup 1.0000x reference)
"""Trainium2 Bass kernel for nn_EqvTransformer (dense_transformer).

Sharding: 8 cores = 4 batches x 2 query-halves. Each core computes the full
attention output for its (batch, 512-query slice) over all 1024 keys and all
8 heads; no cross-core communication (fc_o is row-local).

Layout: "transposed attention" - logits are built as l^T[k, q] tiles (keys on
partitions, queries free):
  - content logits via a 65-row contract: rows 0-63 are K^T/Q^T for the head,
    row 64 is (ones | -1e30*(1-p_q)) so the absent-query mask rides the same
    matmul for free,
  - the pairwise-MLP location logits are accumulated into the same PSUM tile
    by identity-stationary matmuls (PE adds tiles for free),
  - exp() evacuation applies the key-mask + b2[h] as a per-partition ACT bias,
  - the softmax denominator falls out of the A.V matmul via a ones-column
    appended to V.

Pairwise MLP (the arithmetic bottleneck) is split across engines; per hidden
unit, normalize by the largest |W1| coefficient p (ratios <= 1 so bf16 stays
accurate). Route A (DVE, perf-mode friendly; scalar_tensor_tensor is avoided
since it supports no DVE perf modes):
    t1 = x_c1 * r1 + b'      (tensor_scalar, 4x mode)
    t2 = x_c2 * r2           (tensor_scalar, 4x mode)
    z  = x_p + t1; z += t2   (tensor_tensor, 2x mode)
    r  = max/min(z, 0) * (a*W2)  (tensor_scalar, 4x mode; sign of a folds in)
then PSUM += I.r on the PE. Route B (zero DVE; N_ROUTE_B units): the PE
assembles z-hat in PSUM via scaled-diagonal stationaries and the scalar
engine relu-evacuates (scale/bias fold the denormalization, b1 and |W2|);
accumulation uses a shared +/-identity stationary. One positive-W2 route-B
unit per head is the "first writer": its relu evac writes the logits PSUM
directly, eliminating its accumulation matmul.

Absent queries (p_q=0): reference yields uniform A over ALL keys, i.e.
Oh[q] = mean(V). The -1e30 row zeroes the column, the denominator is fixed
(s += 1-p_q), and (1-p_q)*mean(V) is added back before fc_o.
"""

import sys, os

sys.path.insert(0, "/opt/trn_rl_repo")

import numpy as np
import ml_dtypes

import concourse.bass as bass
import concourse.tile as tile
from concourse import bacc, mybir
from concourse import bass_utils

B, N, D, H = 4, 1024, 512, 8
HD = D // H          # 64
NQ = 512             # queries per core
NKC = N // 128       # 8 key chunks of 128
NDT = D // 128       # 4 dout tiles of 128
KHALF = 2            # key halves for the MLP tiling
KCH = NKC // KHALF   # 4 key chunks per half
BIGNEG = -1.0e30

F32 = mybir.dt.float32
F32R = mybir.dt.float32r
BF16 = mybir.dt.bfloat16
AF = mybir.ActivationFunctionType
OP = mybir.AluOpType
BF16NP = ml_dtypes.bfloat16

# Units (h, o) routed through PE/ACT (z assembled in PSUM by identity
# matmuls, relu-evacuated by the scalar engine) instead of pure DVE.
N_ROUTE_B = int(os.environ.get("KERNEL_NB", "6"))
USE_DVE2X = int(os.environ.get("KERNEL_DVE2X", "1"))


# ---------------------------------------------------------------------------
# Custom fused DVE ops with hand-written 2X_1PORT uop programs (the repo's
# custom-DVE framework generates 1x programs only; the 2x slot + byte-36
# perf_max plumbing exist but are unused — see 05-custom-dve-design.md "T1").
# Route A's 5-op chain collapses to 2 fused ops at 2 elems/cycle:
#   MAA: u = (x1*r1 + bn) + xp              (STT shape: s1 literal)
#   RMX: r = max(x2*r2 + u, 0) * (a*w2)     (TTSS shape: imm2 = a*w2)
#   RMN: min-variant for pivots with a < 0
# ---------------------------------------------------------------------------
def _register_dve2x():
    from dataclasses import dataclass
    from concourse import dve_ops as DO
    from concourse.dve_spec import (
        Spec, Src0, Src1, C0, C1, C2, Zero, maxx, minn, lower,
    )
    from concourse.dve_spec import _has_src1 as has_src1
    from concourse.dve_uop import (
        UopConfig, UopDpConfig, DveOpSpec, InpSel, AluInp, OutSel, OutPath,
        DelayInp, Trigger,
    )

    def steady(u):
        u.require_inp0 = 1
        u.require_inp1 = 1
        u.trigger = (Trigger.SRC_TENSOR_DONE, Trigger.NONE, Trigger.NONE)
        u.next_uop = (0, 0, 0)
        u.force_two_data_zero = 1
        u.force_two_data_one = 1
        return u

    def dp(stage_ops):
        """stage_ops: list of (op, a, b, passes, captures) per stage."""
        cfg = []
        for op, a, b, passes, caps in stage_ops:
            c = UopDpConfig().enable_alu(op, a, b)
            if passes:
                c.pass_through_delay(*passes)
            for src, lane in caps:
                c.enable_delay_from_src(src, lane)
            cfg.append(c)
        return cfg

    OP = mybir.AluOpType
    PAO, PD = AluInp.PREV_ALU_OUT, DelayInp.PREV_ALU_OUT
    D0, D1, D2, D3, D4, D5 = (AluInp.PREV_DELAY_0, AluInp.PREV_DELAY_1,
                              AluInp.PREV_DELAY_2, AluInp.PREV_DELAY_3,
                              AluInp.PREV_DELAY_4, AluInp.PREV_DELAY_5)
    ALU = {"mul": DO.AluOp.MULTIPLY, "add": DO.AluOp.ADD,
           "max": DO.AluOp.MAX, "min": DO.AluOp.MIN,
           "byp": DO.AluOp.BYPASS}

    def maa_2x():
        # u = (Src0*C0 + C1) + Src1, elem0 on stages 0-2, elem1 on 3-5.
        # inp0 slot feeds stage0 PREV_ALU_OUT (= Src0, saves a lane); v3 has
        # 6 delay lanes: 0=C0 1=C1 2=SRC_1 3=SRC_0_HI 4=SRC_1_HI 5=elem0-out
        u = UopConfig()
        u.enable_input(InpSel.SRC_0, 0).enable_input(InpSel.CONST_0, 1)
        u.enable_input(InpSel.CONST_1, 2).enable_input(InpSel.SRC_1, 3)
        u.enable_input(InpSel.SRC_0_HI, 4).enable_input(InpSel.SRC_1_HI, 5)
        u.datapath_config = dp([
            (ALU["mul"], PAO, D0, [0, 1, 2, 3, 4], []),
            (ALU["add"], PAO, D1, [0, 1, 2, 3, 4], []),
            (ALU["add"], PAO, D2, [0, 1, 3, 4], []),
            (ALU["mul"], D3, D0, [1, 4], [(PD, 5)]),
            (ALU["add"], PAO, D1, [4, 5], []),
            (ALU["add"], PAO, D4, [5], []),
            (ALU["byp"], PAO, PAO, [5], []),
            (ALU["byp"], PAO, PAO, [5], []),
        ])
        u.enable_output(OutSel.DELAY_5, OutPath.WR0_LO)
        u.enable_output(OutSel.ALU_OUT, OutPath.WR0_HI)
        return steady(u)

    def rm_2x(alu):
        # r = (max|min)(Src0*C0 + Src1, 0), elem0 stages 0-2, elem1 3-5.
        # (the a*w2 scale is folded into the acc-matmul stationary instead.)
        # lanes: 0=C0 1=SRC_1 2=ZERO 3=SRC_0_HI 4=SRC_1_HI 5=elem0-out
        u = UopConfig()
        u.enable_input(InpSel.SRC_0, 0).enable_input(InpSel.CONST_0, 1)
        u.enable_input(InpSel.SRC_1, 2).enable_input(InpSel.ZERO, 3)
        u.enable_input(InpSel.SRC_0_HI, 4).enable_input(InpSel.SRC_1_HI, 5)
        u.datapath_config = dp([
            (ALU["mul"], PAO, D0, [0, 1, 2, 3, 4], []),
            (ALU["add"], PAO, D1, [0, 2, 3, 4], []),
            (alu, PAO, D2, [0, 2, 3, 4], []),
            (ALU["mul"], D3, D0, [2, 4], [(PD, 5)]),
            (ALU["add"], PAO, D4, [2, 5], []),
            (alu, PAO, D2, [5], []),
            (ALU["byp"], PAO, PAO, [5], []),
            (ALU["byp"], PAO, PAO, [5], []),
        ])
        u.enable_output(OutSel.DELAY_5, OutPath.WR0_LO)
        u.enable_output(OutSel.ALU_OUT, OutPath.WR0_HI)
        return steady(u)

    @dataclass(frozen=True)
    class DveOp2X(DO.DveOp):
        build_2x: object = None

        def compile(self, ver):
            key = (self.name, ver)
            spec = DO._COMPILE_CACHE.get(key)
            if spec is None:
                spec = DveOpSpec(
                    name=self.name,
                    opcode=DO.get_dve_sub_opcode(self.name),
                    uops=lower(self.spec, ver=ver),
                    rd1_en=has_src1(self.spec),
                )
                if ver == "v3" and self.build_2x is not None:
                    spec.uops_2x = [self.build_2x()]
                    spec.perf_max = 1
                spec.validate(ver)
                DO._COMPILE_CACHE[key] = spec
            return spec

    import numpy as _np

    def _flat(a, like):
        return _np.asarray(a, _np.float32).reshape(like.shape[0], -1)

    ops = {}
    ops["ANT_MAA2X"] = DveOp2X(
        "ANT_MAA2X",
        Spec(
            body=(Src0 * C0 + C1) + Src1,
            reference=lambda in0, in1, s0, s1, imm2: (
                _flat(in0, in0) * s0 + s1) + _flat(in1, in0),
        ),
        subdim=False, uops_sha={}, build_2x=maa_2x,
    )
    ops["ANT_RMX2X"] = DveOp2X(
        "ANT_RMX2X",
        Spec(
            body=maxx(Src0 * C0 + Src1, Zero),
            reference=lambda in0, in1, s0, s1, imm2: _np.maximum(
                _flat(in0, in0) * s0 + _flat(in1, in0), 0.0),
        ),
        subdim=False, uops_sha={}, build_2x=lambda: rm_2x(ALU["max"]),
    )
    ops["ANT_RMN2X"] = DveOp2X(
        "ANT_RMN2X",
        Spec(
            body=minn(Src0 * C0 + Src1, Zero),
            reference=lambda in0, in1, s0, s1, imm2: _np.minimum(
                _flat(in0, in0) * s0 + _flat(in1, in0), 0.0),
        ),
        subdim=False, uops_sha={}, build_2x=lambda: rm_2x(ALU["min"]),
    )
    for name, op in ops.items():
        if name not in DO._SUB_OPCODE_FOR_NAME:
            DO.OPS.append(op)
            DO.CUSTOM_DVE_SPECS[name] = op.spec
            DO._SUB_OPCODE_FOR_NAME[name] = (
                max(DO._SUB_OPCODE_FOR_NAME.values()) + 1)
            assert DO._SUB_OPCODE_FOR_NAME[name] < 0x20
    return ops


_DVE2X_OPS = _register_dve2x() if USE_DVE2X else None


def _custom_dve_2x(vec, op, out, in0, in1, s0=0.0, s1=0.0, imm2=0.0):
    """Clone of bass.Vector._custom_dve that sets byte-36 perf_max=1 so the
    engine may take the 2X_1PORT uop slot (falls back to 1x when the access
    pattern doesn't qualify)."""
    from concourse import bass_isa
    from concourse.dve_ops import get_dve_sub_opcode
    from concourse.dve_table_gen import dve_ver_for

    b = vec.bass
    if op.name not in b.m.ant_custom_dve_ops:
        b.m.ant_custom_dve_ops = sorted({*b.m.ant_custom_dve_ops, op.name})
    op.compile(dve_ver_for(b.trn_type))
    in1_elementwise = len(in1.shape) > 2
    shape = (bass_isa.CustomDveShape.STT if in1_elementwise
             else bass_isa.CustomDveShape.TTSS)
    isa_opcode = b.isa.Opcode[
        f"NEURON_ISA_TPB_OPCODE_CUSTOM_DVE_ANT_{shape.slot()}"
    ].value

    def lsc(v):
        if isinstance(v, (int, float)):
            return mybir.ImmediateValue(dtype=mybir.dt.float32, value=float(v))
        return vec.lower_ap(v, for_isa=True)

    ins = [vec.lower_ap(in0, for_isa=True, opt=True),
           vec.lower_ap(in1, for_isa=True, opt=True), lsc(s0), lsc(s1)]
    outs = [vec.lower_ap(out, for_isa=True, opt=True)]
    return vec.add_instruction(
        bass_isa.InstCustomDveAnt(
            name=b.get_next_instruction_name(),
            op_name=op.name,
            rd1_en=True,
            subdim=0,
            imm2=imm2,
            shape=shape,
            row=get_dve_sub_opcode(op.name),
            isa_opcode=isa_opcode,
            ins=ins,
            outs=outs,
            perf_max=1,
        )
    )


def _unit_params(W1, b1, W2):
    """Per (h, o): pivot channel, ratios, scales for the normalized MLP."""
    units = []
    for h in range(H):
        for o in range(3):
            w = [float(W1[h, o, c]) for c in range(3)]
            p = int(np.argmax(np.abs(w)))
            a = w[p]
            if a == 0.0:
                a = 1e-30
            c1, c2 = [c for c in range(3) if c != p]
            units.append({
                "h": h, "o": o, "p": p, "c1": c1, "c2": c2,
                "r1": w[c1] / a, "r2": w[c2] / a, "bn": float(b1[h, o]) / a,
                "a": a, "w2": float(W2[h, o]),
            })
    return units


def build_program(W1, b1, W2, b2):
    """Build the SPMD program (same for all 8 cores; per-core data differs).

    W1 (H,3,3), b1 (H,3), W2 (H,3), b2 (H,) are baked into instruction
    immediates (the kernel is compiled per call, so the weights are known).
    """
    W1 = np.asarray(W1, np.float64)
    b1 = np.asarray(b1, np.float64)
    W2 = np.asarray(W2, np.float64)
    units = _unit_params(W1, b1, W2)
    # Route-B units live in the middle heads: at startup the PE is busy with
    # projections (route-A heads keep the DVE productive), and the last heads
    # being route-A keeps the PE tail short.
    route_b = set()
    if N_ROUTE_B:
        # Seed each head's largest positive-W2 unit (first-writer eligible:
        # its relu evac initializes the logits PSUM, saving an acc matmul),
        # then fill up to N_ROUTE_B by |a*w2| magnitude.
        for h in range(H):
            pos = [3 * h + o for o in range(3) if units[3 * h + o]["w2"] > 0]
            if pos and len(route_b) < N_ROUTE_B:
                route_b.add(max(pos, key=lambda i: abs(units[i]["a"] * units[i]["w2"])))
        order = sorted(range(24), key=lambda i: -abs(units[i]["a"] * units[i]["w2"]))
        for i in order:
            if len(route_b) >= N_ROUTE_B:
                break
            route_b.add(i)

    nc = bacc.Bacc("TRN2", target_bir_lowering=False, debug=False, num_devices=8)

    dram = {}

    def din(name, shape, dtype=F32):
        dram[name] = nc.dram_tensor(name, shape, dtype, kind="ExternalInput").ap()
        return dram[name]

    # All large inputs are host-prearranged to partition-major [128, free]
    # layouts so every load is a contiguous 2D DMA (minimal descriptors).
    # Keys are host-permuted per core so the core's own query-half keys come
    # FIRST: yt half 0 doubles as the q-slice (no separate ytq load). Layout
    # [p, (half, dt, q)] keeps each half contiguous (one fat DMA per half).
    t_yt = din("yt", [128, 2 * NDT * NQ], BF16)  # raw Y^T full (proj rhs)
    t_wqt = din("wqt", [128, NDT * D], BF16)  # Wq.T / sqrt(D)
    t_wkt = din("wkt", [128, NDT * D], BF16)
    t_wvt = din("wvt", [128, NDT * D], BF16)
    t_wot = din("wot", [128, NDT * D], BF16)
    t_xt = din("xt", [128, NKC * 3 * NQ], BF16)  # [p, (kc, c, q)]
    t_maskq = din("maskq", [1, NQ], BF16)    # -BIG*(1-p_q) row (bf16)
    t_ones = din("ones1", [1, N], BF16)      # ones row (kt row 64)
    t_expb = din("expb", [128, H * NKC], BF16)  # -BIG*(1-p_k) + b2[h]
    t_ompq = din("ompq", [1, NQ])            # 1 - p_q
    t_ompqb = din("ompqb", [1, NQ], BF16)    # 1 - p_q (bf16, for broadcast)
    t_bias5 = din("bias5", [128, 5 * NDT])   # bq|bk|bv|bo|mv packed
    t_ident = din("ident", [128, 128], BF16)  # identity (acc stationary)
    t_out = nc.dram_tensor("out_t", [D, NQ], BF16, kind="ExternalOutput").ap()

    with tile.TileContext(nc) as tc:
        with (
            tc.tile_pool(name="const", bufs=1) as const,
            tc.tile_pool(name="work", bufs=2) as work,
            tc.tile_pool(name="mlpt", bufs=2) as mlpt,
            tc.tile_pool(name="mlpr", bufs=3) as mlpr,
            tc.tile_pool(name="av", bufs=2) as avp,
            tc.tile_pool(name="outp", bufs=1) as outp,
            tc.tile_pool(name="psA", bufs=2, space="PSUM") as psA,
            tc.tile_pool(name="psL", bufs=(2 if route_b else 4), space="PSUM") as psL,
            tc.tile_pool(name="psO", bufs=2, space="PSUM") as psO,
            tc.tile_pool(name="dram", bufs=1, space="DRAM") as dramp,
        ):
            # ---------------- Phase 0: resident loads ----------------
            # PE warm-up: the first vector op is a tiny memset feeding garbage
            # matmuls so the HAM clock gate reaches K=8/8 before the first
            # projection matmul; warm-fill pairs between projection phases
            # keep it there across DMA-arrival stalls.
            warm_sb = const.tile([128, NQ], BF16)
            nc.vector.memset(warm_sb, 0.0)

            def warm(n):
                for _ in range(n):
                    psw = psA.tile([128, NQ], F32, tag="proj", name="psw")
                    nc.tensor.matmul(psw, warm_sb[:, 0:128], warm_sb,
                                     start=True, stop=True)

            warm(10)
            # ALL loads ride the SP queue: one HW-DGE ring already fans out to
            # all 16 DMA engine-queues, and a single ring gives strict FIFO
            # priority (multi-ring splits let the deepest ring hog HBM).
            # Order = when compute needs each tensor.
            yt_sb = const.tile([128, 2, NDT, NQ], BF16)
            yt_src = t_yt.rearrange("p (th dt q) -> p th dt q", th=2, dt=NDT)
            nc.sync.dma_start(yt_sb[:, 0], yt_src[:, 0])
            ytq_sb = yt_sb[:, 0]  # q-slice view (keys host-permuted)
            w_sb = {}
            w_sb["q"] = const.tile([128, NDT, D], BF16, tag="wq", name="wq_sb")
            nc.sync.dma_start(w_sb["q"], t_wqt.rearrange("p (kt d) -> p kt d", kt=NDT))
            bias5_sb = const.tile([128, 5, NDT], F32)
            nc.sync.dma_start(bias5_sb, t_bias5.rearrange("p (i g) -> p i g", i=5))
            bias_sb = {nm: bias5_sb[:, i, :]
                       for i, nm in enumerate(("q", "k", "v", "o", "mv"))}
            ident_sb = const.tile([128, 128], BF16)
            nc.sync.dma_start(ident_sb, t_ident)
            w_sb["k"] = const.tile([128, NDT, D], BF16, tag="wk", name="wk_sb")
            nc.sync.dma_start(w_sb["k"], t_wkt.rearrange("p (kt d) -> p kt d", kt=NDT))
            nc.sync.dma_start(yt_sb[:, 1], yt_src[:, 1])
            xt_sb = const.tile([128, NKC, 3, NQ], BF16)
            xt_src = t_xt.rearrange("p (kc c q) -> p kc c q", kc=NKC, c=3)
            for kq in range(2):
                nc.sync.dma_start(
                    xt_sb[:, kq * 2:(kq + 1) * 2, :, :],
                    xt_src[:, kq * 2:(kq + 1) * 2, :, :],
                )
            # qt/kt as single tiles [65, H, n]: the maskq / ones rows land as
            # ONE broadcast DMA each instead of 8 trigger-expensive tiny ones.
            qts = const.tile([65, H, NQ], BF16)
            kts = const.tile([65, H, N], BF16)
            qt_h = [qts[:, h, :] for h in range(H)]
            kt_h = [kts[:, h, :] for h in range(H)]
            mq = t_maskq[0:1, :]
            nc.sync.dma_start(
                qts[64:65, :, :],
                bass.AP(tensor=mq.tensor, offset=mq.offset,
                        ap=[[0, 1], [0, H], [1, NQ]]),
            )
            on = t_ones[0:1, :]
            nc.sync.dma_start(
                kts[64:65, :, :],
                bass.AP(tensor=on.tensor, offset=on.offset,
                        ap=[[0, 1], [0, H], [1, N]]),
            )
            w_sb["v"] = const.tile([128, NDT, D], BF16, tag="wv", name="wv_sb")
            nc.sync.dma_start(w_sb["v"], t_wvt.rearrange("p (kt d) -> p kt d", kt=NDT))
            for kq in range(2, 4):
                nc.sync.dma_start(
                    xt_sb[:, kq * 2:(kq + 1) * 2, :, :],
                    xt_src[:, kq * 2:(kq + 1) * 2, :, :],
                )
            src = t_ompqb[0:1, :]
            ompq_bc = const.tile([128, NQ], BF16)
            nc.sync.dma_start(
                ompq_bc,
                bass.AP(tensor=src.tensor, offset=src.offset, ap=[[0, 128], [1, NQ]]),
            )
            expb_sb = const.tile([128, H, NKC], BF16)
            nc.sync.dma_start(expb_sb, t_expb.rearrange("p (h kc) -> p h kc", h=H))
            w_sb["o"] = const.tile([128, NDT, D], BF16, tag="wo", name="wo_sb")
            nc.sync.dma_start(w_sb["o"], t_wot.rearrange("p (kt d) -> p kt d", kt=NDT))
            # Scaled identities for route-B units: diag(r1), diag(r2) for the
            # PSUM z assembly; |W2| folds into the ACT evac so accumulation
            # uses a shared +/-I stationary.
            diag_sb = {}
            negi_sb = None
            if route_b:
                negi_sb = const.tile([128, 128], BF16, tag="negi", name="negi")
                nc.vector.tensor_scalar(negi_sb, ident_sb, -1.0, 0.0, OP.mult, OP.add)
            for i in sorted(route_b):
                u = units[i]
                d1 = const.tile([128, 128], BF16, tag=f"d1_{i}", name=f"d1_{i}")
                d2 = const.tile([128, 128], BF16, tag=f"d2_{i}", name=f"d2_{i}")
                nc.vector.tensor_scalar(d1, ident_sb, float(u["r1"]), 0.0, OP.mult, OP.add)
                nc.vector.tensor_scalar(d2, ident_sb, float(u["r2"]), 0.0, OP.mult, OP.add)
                diag_sb[i] = (d1, d2)
            # route-A acc stationaries carry the (a*w2) scale when the fused
            # 2x DVE ops emit unscaled relu outputs
            diagA_sb = {}
            if _DVE2X_OPS is not None:
                for i in range(3 * H):
                    if i in route_b:
                        continue
                    u = units[i]
                    da = const.tile([128, 128], BF16, tag=f"da_{i}", name=f"da_{i}")
                    nc.vector.tensor_scalar(
                        da, ident_sb, float(u["a"] * u["w2"]), 0.0,
                        OP.mult, OP.add,
                    )
                    diagA_sb[i] = da

            # ---------------- Phase 1: projections ----------------
            # qt_h: [65, NQ] bf16 per head (rows 0-63 Q^T/sqrt(D), row 64 maskq)
            # kt_h: [65, N]  bf16 per head (rows 0-63 K^T, row 64 ones)
            for dt in range(NDT):
                ps = psA.tile([128, NQ], F32, tag="proj")
                for kt in range(NDT):
                    nc.tensor.matmul(
                        ps,
                        (w_sb["q"][:, kt, dt * 128:(dt + 1) * 128]),
                        (ytq_sb[:, kt, :]),
                        start=(kt == 0), stop=(kt == NDT - 1),
                    )
                for hh in range(2):
                    nc.scalar.activation(
                        qt_h[2 * dt + hh][0:64, :], ps[64 * hh:64 * hh + 64, :],
                        AF.Identity, bias=bias_sb["q"][64 * hh:64 * hh + 64, dt:dt + 1],
                    )
            warm(3)
            for dt in range(NDT):
                for th in range(2):
                    ps2 = psA.tile([128, NQ], F32, tag="proj")
                    for kt in range(NDT):
                        nc.tensor.matmul(
                            ps2,
                            (w_sb["k"][:, kt, dt * 128:(dt + 1) * 128]),
                            (yt_sb[:, th, kt, :]),
                            start=(kt == 0), stop=(kt == NDT - 1),
                        )
                    for hh in range(2):
                        nc.scalar.activation(
                            kt_h[2 * dt + hh][0:64, th * NQ:(th + 1) * NQ],
                            ps2[64 * hh:64 * hh + 64, :],
                            AF.Identity,
                            bias=bias_sb["k"][64 * hh:64 * hh + 64, dt:dt + 1],
                        )
            warm(3)
            # V natural [token-part, dout-free] (bf16, with ones column per head)
            v_sb = const.tile([128, NKC, H, HD + 1], BF16)
            nc.vector.memset(v_sb[:, :, :, HD:HD + 1], 1.0)
            for tt in range(NKC):
                ps = psA.tile([128, D], F32, tag="proj")
                for kt in range(NDT):
                    nc.tensor.matmul(
                        ps,
                        (yt_sb[:, tt // KCH, kt,
                               (tt % KCH) * 128:(tt % KCH + 1) * 128]),
                        (w_sb["v"][:, kt, :]),
                        start=(kt == 0), stop=(kt == NDT - 1),
                    )
                nc.scalar.activation(
                    v_sb[:, tt, :, 0:HD], ps.rearrange("p (h d) -> p h d", h=H),
                    AF.Identity,
                )
            # V^T for the query slice [dout-part, q-free] (residual + fc_o
            # input): emitted inside the head loop after h=0 so the startup
            # PE stream reaches the attention work sooner.
            vtq_sb = const.tile([128, NDT, NQ], BF16)

            def emit_vtq():
                for dt in range(NDT):
                    ps = psA.tile([128, NQ], F32, tag="proj")
                    for kt in range(NDT):
                        nc.tensor.matmul(
                            ps,
                            (w_sb["v"][:, kt, dt * 128:(dt + 1) * 128]),
                            (ytq_sb[:, kt, :]),
                            start=(kt == 0), stop=(kt == NDT - 1),
                        )
                    nc.scalar.activation(
                        vtq_sb[:, dt, :], ps, AF.Identity,
                        bias=bias_sb["v"][:, dt:dt + 1],
                    )

            # ---------------- Phase 2: attention ----------------
            oht_sb = const.tile([128, NDT, NQ], F32R)
            opre_sb = oht_sb
            opre_bf = const.tile([128, NDT, NQ], BF16)  # fc_o rhs (bf16)
            rb_sb = const.tile([128, NDT, NQ], F32)
            ones64_sb = const.tile([1, 64], F32)
            nc.vector.memset(ones64_sb, 1.0)
            rt_tiles = []
            for h in range(H):
                po = psO.tile([HD + 1, NQ], F32, tag="po")
                # A route-B unit with W2 > 0 can have its relu evacuation
                # write the logits PSUM directly (first contribution),
                # saving its +/-I accumulation matmul entirely.
                fw = next((o for o in range(3)
                           if (3 * h + o) in route_b and units[3 * h + o]["w2"] > 0),
                          None)
                for kh in range(KHALF):
                    # --- pairwise MLP for this (head, key-half) ---
                    r_tiles = []
                    for o in range(3):
                        u = units[3 * h + o]
                        i_u = 3 * h + o
                        if o == fw:
                            continue  # emitted inline in the ps loop below
                        xp = xt_sb[:, kh * KCH:(kh + 1) * KCH, u["p"], :]
                        x1 = xt_sb[:, kh * KCH:(kh + 1) * KCH, u["c1"], :]
                        x2 = xt_sb[:, kh * KCH:(kh + 1) * KCH, u["c2"], :]
                        if i_u not in route_b:
                            r = mlpr.tile([128, KCH, NQ], BF16, tag=f"r{o}")
                            if _DVE2X_OPS is not None:
                                ut = mlpt.tile([128, KCH * NQ], BF16, tag="t1")
                                _custom_dve_2x(
                                    nc.vector, _DVE2X_OPS["ANT_MAA2X"],
                                    out=ut, in0=x1, in1=xp,
                                    s0=u["r1"], s1=u["bn"],
                                )
                                _custom_dve_2x(
                                    nc.vector,
                                    _DVE2X_OPS["ANT_RMX2X" if u["a"] > 0
                                               else "ANT_RMN2X"],
                                    out=r, in0=x2, in1=ut,
                                    s0=u["r2"],
                                )
                                r_tiles.append((r, diagA_sb[i_u]))
                                continue
                            else:
                                t1 = mlpt.tile([128, KCH, NQ], BF16, tag="t1")
                                t2 = mlpt.tile([128, KCH, NQ], BF16, tag="t2")
                                nc.vector.tensor_scalar(
                                    t1, x1, u["r1"], u["bn"], OP.mult, OP.add
                                )
                                nc.vector.tensor_scalar(
                                    t2, x2, u["r2"], 0.0, OP.mult, OP.add
                                )
                                nc.vector.tensor_add(t1, t1, xp)
                                nc.vector.tensor_add(t1, t1, t2)
                                nc.vector.tensor_scalar(
                                    r, t1, 0.0, u["a"] * u["w2"],
                                    OP.max if u["a"] > 0 else OP.min, OP.mult,
                                )
                            r_tiles.append((r, ident_sb))
                        else:
                            # route B: zero-DVE. PE assembles z-hat in PSUM via
                            # scaled-diagonal stationaries; ACT relu-evacuates
                            # with scale=a (denormalization) and bias=b1.
                            d1, d2 = diag_sb[i_u]
                            r = mlpr.tile([128, KCH, NQ], BF16, tag=f"r{o}")
                            for j in range(KCH):
                                psz = psL.tile([128, NQ], F32, tag="z")
                                nc.tensor.matmul(psz, ident_sb, xp[:, j, :],
                                                 start=True, stop=False)
                                nc.tensor.matmul(psz, d1, x1[:, j, :],
                                                 start=False, stop=False)
                                nc.tensor.matmul(psz, d2, x2[:, j, :],
                                                 start=False, stop=True)
                                sc = float(u["a"] * abs(u["w2"]))
                                nc.scalar.activation(
                                    r[:, j, :], psz, AF.Relu,
                                    scale=sc, bias=float(sc * u["bn"]),
                                )
                            r_tiles.append(
                                (r, ident_sb if u["w2"] > 0 else negi_sb))
                    # --- MLP accumulation first (no dependency on the
                    # projections), content logits last, then exp + A.V ---
                    ps_tiles = []
                    for j in range(KCH):
                        kc = kh * KCH + j
                        ps = psL.tile([128, NQ], F32, tag="l")
                        if fw is not None:
                            # first-writer: assemble the fw unit's z in PSUM
                            # and relu-evacuate straight into ps
                            u = units[3 * h + fw]
                            d1, d2 = diag_sb[3 * h + fw]
                            xpj = xt_sb[:, kh * KCH + j, u["p"], :]
                            x1j = xt_sb[:, kh * KCH + j, u["c1"], :]
                            x2j = xt_sb[:, kh * KCH + j, u["c2"], :]
                            psz = psL.tile([128, NQ], F32, tag="z")
                            nc.tensor.matmul(psz, ident_sb, xpj,
                                             start=True, stop=False)
                            nc.tensor.matmul(psz, d1, x1j, start=False, stop=False)
                            nc.tensor.matmul(psz, d2, x2j, start=False, stop=True)
                            sc = float(u["a"] * u["w2"])
                            nc.scalar.activation(
                                ps, psz, AF.Relu,
                                scale=sc, bias=float(sc * u["bn"]),
                            )
                        for oi, (r, stat) in enumerate(r_tiles):
                            nc.tensor.matmul(
                                ps, stat, r[:, j, :],
                                start=(oi == 0 and fw is None), stop=False,
                                skip_group_check=(fw is not None),
                            )
                        nc.tensor.matmul(
                            ps,
                            (kt_h[h][:, kc * 128:(kc + 1) * 128]),
                            (qt_h[h]),
                            start=False, stop=True,
                            skip_group_check=(fw is not None),
                        )
                        ps_tiles.append(ps)
                    for j in range(KCH):
                        kc = kh * KCH + j
                        a = avp.tile([128, NQ], BF16, tag="a")
                        nc.scalar.activation(
                            a, ps_tiles[j], AF.Exp, bias=expb_sb[:, h, kc:kc + 1]
                        )
                        nc.tensor.matmul(
                            po, v_sb[:, kc, h, :], a,
                            start=(kc == 0), stop=(kc == NKC - 1),
                        )
                # denominator row (+ (1-p_q) fix, approx reciprocal) + Oh^T evac
                s_sb = work.tile([1, NQ], F32, tag="s", bufs=2)
                nc.vector.tensor_add(s_sb, po[HD:HD + 1, :], ompq_bc[0:1, :])
                rt = work.tile([1, NQ], F32, tag="rden", bufs=2)
                nc.vector.reciprocal_approx_fast(rt, s_sb)
                rt_tiles.append(rt)
                nc.scalar.activation(
                    oht_sb[64 * (h % 2):64 * (h % 2) + 64, h // 2, :], po[0:HD, :],
                    AF.Identity,
                )
                if h == 0:
                    emit_vtq()
                if h % 2 == 1:
                    # heads 2dt/2dt+1 done: assemble OPre[:, dt, :] right away
                    # (OPre = Vq + r*Oh + (1-p_q)*meanV, in place over oht_sb).
                    # Broadcast the two reciprocal rows across 64 partitions
                    # each with rank-1 matmuls (no DRAM bounce).
                    dt = h // 2
                    psb = psA.tile([128, NQ], F32, tag="proj")
                    for hh in range(2):
                        nc.tensor.matmul(
                            psb[64 * hh:64 * hh + 64, :], ones64_sb,
                            rt_tiles[2 * dt + hh],
                            start=True, stop=True, skip_group_check=True,
                        )
                    nc.scalar.activation(rb_sb[:, dt, :], psb, AF.Identity)
                    nc.vector.tensor_mul(
                        opre_sb[:, dt, :], oht_sb[:, dt, :], rb_sb[:, dt, :]
                    )
                    nc.vector.tensor_add(
                        opre_sb[:, dt, :], opre_sb[:, dt, :], vtq_sb[:, dt, :]
                    )
                    nc.vector.scalar_tensor_tensor(
                        opre_bf[:, dt, :], ompq_bc, bias_sb["mv"][:, dt:dt + 1],
                        opre_sb[:, dt, :], OP.mult, OP.add,
                    )

            # ---------------- Phase 3: fc_o ----------------
            for dt in range(NDT):
                ps = psA.tile([128, NQ], F32, tag="proj")
                for kt in range(NDT):
                    nc.tensor.matmul(
                        ps,
                        (w_sb["o"][:, kt, dt * 128:(dt + 1) * 128]),
                        (opre_bf[:, kt, :]),
                        start=(kt == 0), stop=(kt == NDT - 1),
                    )
                relu_sb = outp.tile([128, NQ], BF16, tag="relu", bufs=2)
                nc.scalar.activation(
                    relu_sb, ps, AF.Relu, bias=bias_sb["o"][:, dt:dt + 1]
                )
                nc.vector.tensor_add(relu_sb, relu_sb, opre_bf[:, dt, :])
                nc.sync.dma_start(t_out[dt * 128:(dt + 1) * 128, :], relu_sb)

    nc.compile()
    return nc


def _pm(x, g):
    """(g*128, F) -> partition-major contiguous [128, g*F]."""
    gp, F = x.shape
    return np.ascontiguousarray(
        x.reshape(g, 128, F).transpose(1, 0, 2).reshape(128, g * F))


def _pm1(x):
    """(g*128,) -> [128, g]."""
    g = x.shape[0] // 128
    return np.ascontiguousarray(x.reshape(g, 128).T)


def make_in_maps(inputs):
    """Host-side prep: returns the per-core input dicts."""
    Y = np.asarray(inputs["Y_lift"], np.float32)
    X = np.asarray(inputs["X_pairs"], np.float32)
    pres = np.asarray(inputs["presence"], np.float32)
    Wq = np.asarray(inputs["Wq"], np.float32)
    Wk = np.asarray(inputs["Wk"], np.float32)
    Wv = np.asarray(inputs["Wv"], np.float32)
    Wo = np.asarray(inputs["Wo"], np.float32)
    bq = np.asarray(inputs["bq"], np.float32)
    bk = np.asarray(inputs["bk"], np.float32)
    bv = np.asarray(inputs["bv"], np.float32)
    bo = np.asarray(inputs["bo"], np.float32)
    b2 = np.asarray(inputs["b2"], np.float32)

    inv_sqrt = np.float32(1.0 / np.sqrt(D))
    WqT = np.ascontiguousarray(Wq.T * inv_sqrt)
    WkT = np.ascontiguousarray(Wk.T)
    WvT = np.ascontiguousarray(Wv.T)
    WoT = np.ascontiguousarray(Wo.T)

    Yt = np.ascontiguousarray(Y.transpose(0, 2, 1))            # (B, D, N)
    XT = np.ascontiguousarray(X.transpose(0, 3, 2, 1))          # (B, 3, k, q)
    V_full = Y @ Wv.T + bv                                      # (B, N, D) host
    meanV = V_full.mean(axis=1).astype(np.float32)              # (B, D)
    ident = np.eye(128, dtype=BF16NP)

    # fold 1/sqrt(D) scaling into bq too (Q^T evac bias rides the scaled path)
    bq_s = (bq * inv_sqrt).astype(np.float32)

    in_maps = []
    for c in range(8):
        b, qh = c // 2, c % 2
        qsl = slice(qh * NQ, (qh + 1) * NQ)
        # per-core key permutation: own query-half keys first, so yt columns
        # [0, NQ) double as the Q-projection rhs (attention sums over keys
        # are order-invariant; all key-indexed tensors permute consistently)
        kperm = np.r_[qh * NQ:(qh + 1) * NQ, (1 - qh) * NQ:(2 - qh) * NQ]
        pres_k = pres[b][kperm]
        pkb = (BIGNEG * (1.0 - pres_k)).astype(np.float32)      # (N,)
        expb = (pkb[None, :] + b2[:, None]).astype(BF16NP)      # (H, N)
        # xt: [p, (kc, c, q)] interleaved channel layout (keys permuted)
        xtb = XT[b][:, kperm, :][:, :, qsl].astype(BF16NP)      # (3, N, NQ)
        xtb = xtb.reshape(3, NKC, 128, NQ).transpose(2, 1, 0, 3)
        xtb = np.ascontiguousarray(xtb.reshape(128, NKC * 3 * NQ))
        # expb: [p, (h, kc)]
        expb_pm = np.ascontiguousarray(
            expb.reshape(H, NKC, 128).transpose(2, 0, 1).reshape(128, H * NKC))
        bias5 = np.concatenate(
            [_pm1(bq_s), _pm1(bk), _pm1(2.0 * bv), _pm1(bo),
             _pm1(meanV[b] - bv)], axis=1)
        # yt: [p, (half, dt, q)] — each key-half contiguous
        ytp = _pm(np.ascontiguousarray(Yt[b][:, kperm]), NDT)   # [128, (dt n)]
        ytp = ytp.reshape(128, NDT, 2, NQ).transpose(0, 2, 1, 3)
        in_maps.append({
            "yt": np.ascontiguousarray(ytp.reshape(128, 2 * NDT * NQ)).astype(BF16NP),
            "wqt": _pm(WqT, NDT).astype(BF16NP), "wkt": _pm(WkT, NDT).astype(BF16NP),
            "wvt": _pm(WvT, NDT).astype(BF16NP),
            "wot": _pm(WoT, NDT).astype(BF16NP),
            "xt": xtb,
            "maskq": (BIGNEG * (1.0 - pres[b, qsl])).astype(BF16NP).reshape(1, NQ),
            "expb": expb_pm,
            "ompq": (1.0 - pres[b, qsl]).astype(np.float32).reshape(1, NQ),
            "ompqb": (1.0 - pres[b, qsl]).astype(BF16NP).reshape(1, NQ),
            "ones1": np.ones((1, N), BF16NP),
            "bias5": np.ascontiguousarray(bias5),
            "ident": ident,
        })
    return in_maps


def assemble_output(results):
    out = np.empty((B, N, D), np.float32)
    for c in range(8):
        b, qh = c // 2, c % 2
        out[b, qh * NQ:(qh + 1) * NQ, :] = results[c]["out_t"].T.astype(np.float32)
    return out


def kernel(**inputs):
    nc = build_program(inputs["W1"], inputs["b1"], inputs["W2"], inputs["b2"])
    in_maps = make_in_maps(inputs)
    trace = bool(int(os.environ.get("KERNEL_TRACE", "0")))
    res = bass_utils.run_bass_kernel_spmd(
        nc, in_maps, core_ids=list(range(8)), trace=trace
    )
    kernel.last_result = res
    return assemble_output(res.results)

